# revision 32
# baseline (speedup 1.0000x reference)
"""Distributed Trainium2 kernel for a single causal attention head.

Module: k,q,v = x@W{k,q,v}.T ; a = softmax(causal(q@k.T/sqrt(64))) ; out = a@v
Shapes: x (4, 4096, 1024) f32; W* (64, 1024) f32; out (4, 4096, 64) f32.

Sharding (one SPMD launch, 8 cores, no collectives): 4 batches x 2
key-parity halves. Core c: batch b=c//2, parity p=c%2. The 32 key chunks
(128 tokens) of a batch are split by parity (even chunks -> p=0, odd ->
p=1), which makes the causal work *and* the instruction structure
identical on every core: for query chunk j (512 tokens), each core
processes exactly 2j+2 of its local key chunks; its two diagonal mask
tiles arrive as input data. To keep all SBUF addresses SPMD-uniform, the
host hands each core x[b].T with token columns permuted so the core's
own-parity key blocks sit at even 128-block positions (identity for p=0,
adjacent-block swap for p=1).

Per core: project K^T,V^T (packed [Wk|Wv]) for own-parity tokens and Q^T
([Wq|0]) for ALL tokens; V^T -> V by PE transpose (ones column appended
-> softmax sums ride along row 64 of the AV output); per (qchunk 512,
local kchunk pair 256): S^T = K^T.T @ Q^T, P^T = exp(S^T/8) on ACT over
the 1024-wide pair (diagonal pair multiplied by the input mask), then
O'^T(65,512) += [V|1].T @ P^T. The partial [O'^T; l] (65, 4096) goes to
DRAM; the host adds the two parity partials per batch, divides by the
summed denominators l, un-permutes and transposes (the standard
partial-softmax combine; no max-subtraction is needed since the logits
are O(1) by construction). All matmuls run in float32r (f32 storage,
11-bit mantissa in the PE at full rate, ~1e-4 matmul error); PSUM
accumulates in f32.
"""

import numpy as np

B, T, E, H = 4, 4096, 1024, 64
P = 128           # partitions
QC = 512          # query chunk (matmul moving free dim)
KC = 128          # key chunk
ETILES = E // P   # 8 contraction tiles
NKCH = T // KC // 2   # 16 local (parity) key chunks per core
NREG = 4          # 1024-column load/projection regions
NQCH = T // QC    # 8 query chunks
TLOC = T // 2     # 2048 local (own-parity) tokens

_CACHE = {}


def _round_f32r(a: np.ndarray) -> np.ndarray:
    """Round f32 to float32r (11 mantissa bits, round-half-up) as the PE
    expects for f32r matmul operands."""
    u = np.ascontiguousarray(a, dtype=np.float32).view(np.uint32)
    r = ((u.astype(np.uint64) + 0x800) & 0xFFFFF000).astype(np.uint32)
    return r.view(np.float32)


COMPUTE = "bf16"  # "f32r" (11-bit mantissa, ~2e-4 err) or "bf16" (~4e-3 err)


def _build_graph():
    import concourse.bass as bass
    import concourse.tile as tile
    from concourse import bacc, mybir
    from concourse.masks import make_identity

    f32 = mybir.dt.float32
    f32r = mybir.dt.float32r if COMPUTE == "f32r" else mybir.dt.bfloat16
    AF = mybir.ActivationFunctionType
    ALU = mybir.AluOpType
    RC = T // NREG  # 1024 columns per region

    nc = bacc.Bacc("TRN2", target_bir_lowering=False, debug=False, num_devices=8)
    xTa_d = nc.dram_tensor("xTa", [E, T], f32r, kind="ExternalInput").ap()
    wkv_d = nc.dram_tensor("wkv", [E, P], f32r, kind="ExternalInput").ap()
    wvk_d = nc.dram_tensor("wvk", [E, P], f32r, kind="ExternalInput").ap()
    wqq_d = nc.dram_tensor("wqq", [E, P], f32r, kind="ExternalInput").ap()
    dmask_d = nc.dram_tensor("dmask", [P, 2, QC], f32r, kind="ExternalInput").ap()
    out_d = nc.dram_tensor("o", [H + 1, NQCH, QC], f32, kind="ExternalOutput").ap()

    with tile.TileContext(nc) as tc:
        with (
            tc.tile_pool(name="consts", bufs=1) as consts,
            tc.tile_pool(name="xin", bufs=3) as xin,
            tc.tile_pool(name="big", bufs=1) as big,
            tc.tile_pool(name="work", bufs=3) as work,
            tc.tile_pool(name="psum", bufs=1, space="PSUM") as psum,
        ):
            # ---- constants ----
            ident32 = consts.tile([P, P], f32)
            make_identity(nc, ident32)
            ident = consts.tile([P, P], f32r)
            nc.vector.tensor_copy(ident[:], ident32[:])
            wkv_sb = consts.tile([P, ETILES, P], f32r)
            nc.sync.dma_start(wkv_sb[:], wkv_d.rearrange("(ko p) m -> p ko m", p=P))
            wvk_sb = consts.tile([P, ETILES, P], f32r)
            nc.sync.dma_start(wvk_sb[:], wvk_d.rearrange("(ko p) m -> p ko m", p=P))
            wqq_sb = consts.tile([P, ETILES, P], f32r)
            nc.sync.dma_start(wqq_sb[:], wqq_d.rearrange("(ko p) m -> p ko m", p=P))
            dmask_sb = consts.tile([P, 2, QC], f32r)
            nc.sync.dma_start(dmask_sb[:], dmask_d[:])
            ones32 = consts.tile([P, 1], f32)
            nc.vector.memset(ones32[:], 1.0)

            # ---- projections ----
            # kvA: [K^T; V^T] for even local key chunks (local 2m)
            # kvB: [V^T; K^T] for odd  local key chunks (local 2m+1)
            # -> K lives on partitions 0:64 for A and 64:128 for B, enabling
            #    row-packed S^T (both PE array halves run concurrently)
            kvA = big.tile([P, TLOC // 2], f32r)
            kvB = big.tile([P, TLOC // 2], f32r)
            q_all = big.tile([P, T], f32r)      # [Q^T; Q^T], all tokens
            v_sb = big.tile([P, NKCH, H + 1], f32r)
            nc.vector.tensor_copy(v_sb[:, :, H:H + 1],
                                  ones32[:, None, :].to_broadcast((P, NKCH, 1)))

            for r in range(NREG):
                xt = xin.tile([P, ETILES, RC], f32r, tag="xt")
                for ko in range(ETILES):
                    nc.sync.dma_start(
                        xt[:, ko],
                        xTa_d[ko * P:(ko + 1) * P, r * RC:(r + 1) * RC])
                # Q for both 512-chunks of the region
                for half in range(2):
                    pq = psum.tile([P, QC], f32, tag="proj", bufs=2)
                    for ko in range(ETILES):
                        nc.tensor.matmul(pq[:], wqq_sb[:, ko],
                                         xt[:, ko, half * QC:(half + 1) * QC],
                                         start=(ko == 0), stop=(ko == ETILES - 1))
                    c = r * RC + half * QC
                    nc.vector.tensor_copy(q_all[:, c:c + QC], pq[:])
                # compact the region's own-parity blocks (positions 0,2,4,6)
                # into A-chunks (positions 0,4 -> local 4r,4r+2) and B-chunks
                # (positions 2,6 -> local 4r+1,4r+3); PE needs contiguous
                # moving operands (strided rhs crashes the device)
                xab = work.tile([P, ETILES, 2, 2 * KC], f32r, tag="xab", bufs=2)
                for ko in range(ETILES):
                    blk = xt[:, ko].rearrange("p (a b c) -> p a b c", b=4, c=KC)
                    nc.vector.tensor_copy(
                        xab[:, ko, 0].rearrange("p (a c) -> p a c", c=KC),
                        blk[:, :, 0, :])
                    nc.vector.tensor_copy(
                        xab[:, ko, 1].rearrange("p (a c) -> p a c", c=KC),
                        blk[:, :, 2, :])
                pab = psum.tile([P, 2, 2 * KC], f32, tag="proj", bufs=2)
                for ko in range(ETILES):
                    nc.tensor.matmul(pab[:, 0], wkv_sb[:, ko], xab[:, ko, 0],
                                     start=(ko == 0), stop=(ko == ETILES - 1))
                for ko in range(ETILES):
                    nc.tensor.matmul(pab[:, 1], wvk_sb[:, ko], xab[:, ko, 1],
                                     start=(ko == 0), stop=(ko == ETILES - 1))
                nc.vector.tensor_copy(kvA[:, r * 2 * KC:(r + 1) * 2 * KC],
                                      pab[:, 0])
                nc.vector.tensor_copy(kvB[:, r * 2 * KC:(r + 1) * 2 * KC],
                                      pab[:, 1])
                # V^T -> V for the region's 4 local key chunks
                for w in range(2):
                    for u in range(2):
                        i = 4 * r + 2 * u + w  # local chunk; w=0 even(A), w=1 odd(B)
                        src = (kvA if w == 0 else kvB)
                        cols = (2 * r + u) * KC
                        ptr = psum.tile([P, P], f32r, tag="ptr", bufs=1,
                                        name=f"ptr_{i}")
                        nc.tensor.transpose(ptr[:], src[:, cols:cols + KC],
                                            ident[:])
                        vcols = slice(H, P) if w == 0 else slice(0, H)
                        nc.vector.tensor_copy(v_sb[:, i, 0:H], ptr[:, vcols])

            # ---- attention (partial, own-parity keys) ----
            # j=0 (shortest) goes last so the final drain chain is short
            for j in list(range(1, NQCH)) + [0]:
                npair = j + 1  # local kchunk pairs; extent = 2j+2 chunks
                po = psum.tile([H + 1, QC], f32, tag="po", bufs=1, name=f"po_{j}")
                qs = q_all[:, j * QC:(j + 1) * QC]

                def s_pair(m):
                    # row-packed: even chunk on PE rows 0:63, odd on 64:127
                    ps = psum.tile([P, 2, QC], f32, tag="ps", bufs=2,
                                   name=f"ps_{j}_{m}")
                    nc.tensor.matmul(ps[:, 0], kvA[0:H, m * KC:(m + 1) * KC],
                                     qs[0:H, :], start=True, stop=True,
                                     tile_position=(0, 0))
                    nc.tensor.matmul(ps[:, 1], kvB[H:P, m * KC:(m + 1) * KC],
                                     qs[H:P, :], start=True, stop=True,
                                     tile_position=(64, 0))
                    return ps

                def exp_pair(m, ps):
                    pt = work.tile([P, 2, QC], f32r, tag="pt", bufs=3,
                                   name=f"pt_{j}_{m}")
                    nc.scalar.activation(pt[:], ps[:], AF.Exp,
                                         scale=float(H) ** -0.5)
                    if m == j:  # diagonal pair
                        nc.vector.tensor_tensor(pt[:], pt[:], dmask_sb[:],
                                                ALU.mult)
                    return pt

                def av_pair(m, pt, first, last):
                    for u in range(2):
                        i = 2 * m + u
                        nc.tensor.matmul(po[:], v_sb[:, i, :], pt[:, u],
                                         start=(first and u == 0),
                                         stop=(last and u == 1))

                # diagonal (masked) pair first so the DVE mask never gates the
                # final AV; software-pipelined emission: S(next) before AV(cur)
                order = [j] + list(range(j))
                ps = s_pair(order[0])
                pt = exp_pair(order[0], ps)
                for idx in range(1, npair):
                    ps2 = s_pair(order[idx])
                    av_pair(order[idx - 1], pt, idx - 1 == 0, False)
                    pt = exp_pair(order[idx], ps2)
                av_pair(order[-1], pt, npair == 1, True)

                ost = work.tile([H + 1, QC], f32, tag="ost", bufs=2)
                nc.vector.tensor_copy(ost[:], po[:])
                nc.sync.dma_start(out_d[:, j], ost[:])

    nc.compile()
    return nc


def _get_graph():
    if "g" not in _CACHE:
        _CACHE["g"] = _build_graph()
    return _CACHE["g"]


def _perm(p: int) -> np.ndarray:
    """Token column permutation for parity p: own-parity 128-blocks at even
    block positions (identity for p=0, adjacent-block swap for p=1)."""
    blocks = np.arange(T // KC).reshape(-1, 2)
    if p == 1:
        blocks = blocks[:, ::-1]
    return (blocks.reshape(-1)[:, None] * KC + np.arange(KC)[None, :]).reshape(-1)


def _make_masks(p: int) -> np.ndarray:
    """Diagonal-pair masks in permuted column space: column t' of a query
    chunk is global token offset sigma(t'); diag chunks have global key
    offsets 128*p (slot 0) and 128*(p+2) (slot 1) within the chunk."""
    perm = _perm(p)
    sigma = perm[:QC] % QC  # within-chunk token offset pattern (j-independent)
    s = np.arange(P)[:, None]
    m = np.empty((P, 2, QC), np.float32)
    m[:, 0] = (sigma[None, :] - s - KC * p) >= 0
    m[:, 1] = (sigma[None, :] - s - KC * (p + 2)) >= 0
    return m


def _run(x, Wk, Wq, Wv, trace=False):
    from concourse.bass_utils import run_bass_kernel_spmd

    x = np.asarray(x, dtype=np.float32)
    Wk = np.asarray(Wk, dtype=np.float32)
    Wq = np.asarray(Wq, dtype=np.float32)
    Wv = np.asarray(Wv, dtype=np.float32)

    if COMPUTE == "f32r":
        conv = _round_f32r
    else:
        import ml_dtypes
        conv = lambda a: np.asarray(a, dtype=ml_dtypes.bfloat16)
    wkv = conv(np.concatenate([Wk.T, Wv.T], axis=1))
    wvk = conv(np.concatenate([Wv.T, Wk.T], axis=1))
    wqq = conv(np.concatenate([Wq.T, Wq.T], axis=1))
    masks = [conv(_make_masks(0)), conv(_make_masks(1))]
    perms = [_perm(0), _perm(1)]

    in_maps = []
    xTb = {}
    for c in range(8):
        b, p = c // 2, c % 2
        if (b, p) not in xTb:
            xTb[(b, p)] = conv(x[b].T[:, perms[p]])
        in_maps.append({"xTa": xTb[(b, p)], "wkv": wkv, "wvk": wvk, "wqq": wqq,
                        "dmask": masks[p]})

    nc = _get_graph()
    res = run_bass_kernel_spmd(nc, in_maps, core_ids=list(range(8)), trace=trace)

    out = np.empty((B, T, H), dtype=np.float32)
    for b in range(B):
        o0 = res.results[2 * b]["o"].reshape(H + 1, T)
        o1 = res.results[2 * b + 1]["o"].reshape(H + 1, T)
        # p=1 columns are block-swapped; un-permute before merging
        o1 = o1[:, perms[1]]
        s = o0 + o1
        out[b] = (s[0:H] / s[H:H + 1]).T
    return out, res.exec_time_ns


def kernel(x, Wk, Wq, Wv):
    out, _ = _run(x, Wk, Wq, Wv)
    return out


# revision 33
# speedup vs baseline: 1.1897x; 1.1897x over previous
"""Distributed Trainium2 kernel for a single causal attention head.

Module: k,q,v = x@W{k,q,v}.T ; a = softmax(causal(q@k.T/sqrt(64))) ; out = a@v
Shapes: x (4, 4096, 1024) f32; W* (64, 1024) f32; out (4, 4096, 64) f32.

Sharding (one SPMD launch, 8 cores, no collectives): 4 batches x 2
key-parity halves. Core c: batch b=c//2, parity p=c%2. The 32 key chunks
(128 tokens) of a batch are split by parity (even chunks -> p=0, odd ->
p=1), which makes the causal work *and* the instruction structure
identical on every core: for query chunk j (512 tokens), each core
processes exactly 2j+2 of its local key chunks; its two diagonal mask
tiles arrive as input data. To keep all SBUF addresses SPMD-uniform, the
host hands each core x[b].T with token columns permuted so the core's
own-parity key blocks sit at even 128-block positions (identity for p=0,
adjacent-block swap for p=1).

Per core: project K^T,V^T (packed [Wk|Wv]) for own-parity tokens and Q^T
([Wq|0]) for ALL tokens; V^T -> V by PE transpose (ones column appended
-> softmax sums ride along row 64 of the AV output); per (qchunk 512,
local kchunk pair 256): S^T = K^T.T @ Q^T, P^T = exp(S^T/8) on ACT over
the 1024-wide pair (diagonal pair multiplied by the input mask), then
O'^T(65,512) += [V|1].T @ P^T. The partial [O'^T; l] (65, 4096) goes to
DRAM; the host adds the two parity partials per batch, divides by the
summed denominators l, un-permutes and transposes (the standard
partial-softmax combine; no max-subtraction is needed since the logits
are O(1) by construction). All matmuls run in float32r (f32 storage,
11-bit mantissa in the PE at full rate, ~1e-4 matmul error); PSUM
accumulates in f32.
"""

import numpy as np

B, T, E, H = 4, 4096, 1024, 64
P = 128           # partitions
QC = 512          # query chunk (matmul moving free dim)
KC = 128          # key chunk
ETILES = E // P   # 8 contraction tiles
NKCH = T // KC // 2   # 16 local (parity) key chunks per core
NREG = 4          # 1024-column load/projection regions
NQCH = T // QC    # 8 query chunks
TLOC = T // 2     # 2048 local (own-parity) tokens

_CACHE = {}


def _round_f32r(a: np.ndarray) -> np.ndarray:
    """Round f32 to float32r (11 mantissa bits, round-half-up) as the PE
    expects for f32r matmul operands."""
    u = np.ascontiguousarray(a, dtype=np.float32).view(np.uint32)
    r = ((u.astype(np.uint64) + 0x800) & 0xFFFFF000).astype(np.uint32)
    return r.view(np.float32)


COMPUTE = "bf16"  # "f32r" (11-bit mantissa, ~2e-4 err) or "bf16" (~4e-3 err)


def _build_graph():
    import concourse.bass as bass
    import concourse.tile as tile
    from concourse import bacc, mybir
    from concourse.masks import make_identity

    f32 = mybir.dt.float32
    f32r = mybir.dt.float32r if COMPUTE == "f32r" else mybir.dt.bfloat16
    AF = mybir.ActivationFunctionType
    ALU = mybir.AluOpType
    RC = T // NREG  # 1024 columns per region

    nc = bacc.Bacc("TRN2", target_bir_lowering=False, debug=False, num_devices=8)
    xTa_d = nc.dram_tensor("xTa", [E, T], f32r, kind="ExternalInput").ap()
    wkv_d = nc.dram_tensor("wkv", [E, P], f32r, kind="ExternalInput").ap()
    wq0_d = nc.dram_tensor("wq0", [E, P], f32r, kind="ExternalInput").ap()
    dmask_d = nc.dram_tensor("dmask", [P, 2, QC], f32r, kind="ExternalInput").ap()
    out_d = nc.dram_tensor("o", [H + 1, NQCH, QC], f32, kind="ExternalOutput").ap()

    with tile.TileContext(nc) as tc:
        with (
            tc.tile_pool(name="consts", bufs=1) as consts,
            tc.tile_pool(name="xin", bufs=3) as xin,
            tc.tile_pool(name="big", bufs=1) as big,
            tc.tile_pool(name="work", bufs=3) as work,
            tc.tile_pool(name="psum", bufs=1, space="PSUM") as psum,
        ):
            # ---- constants ----
            ident32 = consts.tile([P, P], f32)
            make_identity(nc, ident32)
            ident = consts.tile([P, P], f32r)
            nc.vector.tensor_copy(ident[:], ident32[:])
            wkv_sb = consts.tile([P, ETILES, P], f32r)
            nc.sync.dma_start(wkv_sb[:], wkv_d.rearrange("(ko p) m -> p ko m", p=P))
            wq0_sb = consts.tile([P, ETILES, P], f32r)
            nc.sync.dma_start(wq0_sb[:], wq0_d.rearrange("(ko p) m -> p ko m", p=P))
            dmask_sb = consts.tile([P, 2, QC], f32r)
            nc.sync.dma_start(dmask_sb[:], dmask_d[:])
            zeros32 = consts.tile([H, 1], f32)
            nc.vector.memset(zeros32[:], 0.0)
            ones32 = consts.tile([P, 1], f32)
            nc.vector.memset(ones32[:], 1.0)

            # ---- projections ----
            kv_sb = big.tile([P, TLOC], f32r)   # [K^T; V^T], own-parity keys
            q_all = big.tile([P, T], f32r)      # [Q^T; 0], all tokens
            nc.vector.tensor_copy(q_all[H:P, :],
                                  zeros32[:, :].to_broadcast((H, T)))
            v_sb = big.tile([P, NKCH, H + 1], f32r)
            nc.vector.tensor_copy(v_sb[:, :, H:H + 1],
                                  ones32[:, None, :].to_broadcast((P, NKCH, 1)))

            for r in range(NREG):
                xt = xin.tile([P, ETILES, RC], f32r, tag="xt")
                for ko in range(ETILES):
                    nc.sync.dma_start(
                        xt[:, ko],
                        xTa_d[ko * P:(ko + 1) * P, r * RC:(r + 1) * RC])
                # Q for both 512-chunks of the region
                for half in range(2):
                    pq = psum.tile([P, QC], f32, tag="proj", bufs=2)
                    for ko in range(ETILES):
                        nc.tensor.matmul(pq[:], wq0_sb[:, ko],
                                         xt[:, ko, half * QC:(half + 1) * QC],
                                         start=(ko == 0), stop=(ko == ETILES - 1))
                    c = r * RC + half * QC
                    nc.vector.tensor_copy(q_all[:, c:c + QC], pq[:])
                # K,V for the region's even (own-parity) 128-blocks.
                # The PE crashes on strided moving operands, so compact the
                # even blocks into a contiguous tile on DVE first.
                xkv = work.tile([P, ETILES, QC], f32r, tag="xkv", bufs=2)
                for ko in range(ETILES):
                    nc.vector.tensor_copy(
                        xkv[:, ko],
                        xt[:, ko].rearrange("p (u v c) -> p u v c",
                                            v=2, c=KC)[:, :, 0, :])
                pkv = psum.tile([P, QC], f32, tag="proj", bufs=2)
                for ko in range(ETILES):
                    nc.tensor.matmul(pkv[:], wkv_sb[:, ko], xkv[:, ko],
                                     start=(ko == 0), stop=(ko == ETILES - 1))
                nc.vector.tensor_copy(kv_sb[:, r * QC:(r + 1) * QC], pkv[:])
                # V^T -> V for the region's 4 local key chunks
                for i in range(4 * r, 4 * r + 4):
                    ptr = psum.tile([P, P], f32r, tag="ptr", bufs=1,
                                    name=f"ptr_{i}")
                    nc.tensor.transpose(ptr[:], kv_sb[:, i * KC:(i + 1) * KC],
                                        ident[:])
                    nc.vector.tensor_copy(v_sb[:, i, 0:H], ptr[:, H:P])

            # ---- attention (partial, own-parity keys) ----
            # j=0 (shortest) goes last so the final drain chain is short
            for j in list(range(1, NQCH)) + [0]:
                npair = j + 1  # local kchunk pairs; extent = 2j+2 chunks
                po = psum.tile([H + 1, QC], f32, tag="po", bufs=1, name=f"po_{j}")
                qs = q_all[:, j * QC:(j + 1) * QC]

                def s_pair(m):
                    ps = psum.tile([P, 2, QC], f32, tag="ps", bufs=2,
                                   name=f"ps_{j}_{m}")
                    for u in range(2):
                        i = 2 * m + u
                        nc.tensor.matmul(ps[:, u], kv_sb[:, i * KC:(i + 1) * KC],
                                         qs, start=True, stop=True)
                    return ps

                def exp_pair(m, ps):
                    pt = work.tile([P, 2, QC], f32r, tag="pt", bufs=3,
                                   name=f"pt_{j}_{m}")
                    nc.scalar.activation(pt[:], ps[:], AF.Exp,
                                         scale=float(H) ** -0.5)
                    if m == j:  # diagonal pair
                        nc.vector.tensor_tensor(pt[:], pt[:], dmask_sb[:],
                                                ALU.mult)
                    return pt

                def av_pair(m, pt, first, last):
                    for u in range(2):
                        i = 2 * m + u
                        nc.tensor.matmul(po[:], v_sb[:, i, :], pt[:, u],
                                         start=(first and u == 0),
                                         stop=(last and u == 1))

                # diagonal (masked) pair first so the DVE mask never gates the
                # final AV; software-pipelined emission: S(next) before AV(cur)
                order = [j] + list(range(j))
                ps = s_pair(order[0])
                pt = exp_pair(order[0], ps)
                for idx in range(1, npair):
                    ps2 = s_pair(order[idx])
                    av_pair(order[idx - 1], pt, idx - 1 == 0, False)
                    pt = exp_pair(order[idx], ps2)
                av_pair(order[-1], pt, npair == 1, True)

                ost = work.tile([H + 1, QC], f32, tag="ost", bufs=2)
                nc.vector.tensor_copy(ost[:], po[:])
                nc.sync.dma_start(out_d[:, j], ost[:])

    nc.compile()
    return nc


def _get_graph():
    if "g" not in _CACHE:
        _CACHE["g"] = _build_graph()
    return _CACHE["g"]


def _perm(p: int) -> np.ndarray:
    """Token column permutation for parity p: own-parity 128-blocks at even
    block positions (identity for p=0, adjacent-block swap for p=1)."""
    blocks = np.arange(T // KC).reshape(-1, 2)
    if p == 1:
        blocks = blocks[:, ::-1]
    return (blocks.reshape(-1)[:, None] * KC + np.arange(KC)[None, :]).reshape(-1)


def _make_masks(p: int) -> np.ndarray:
    """Diagonal-pair masks in permuted column space: column t' of a query
    chunk is global token offset sigma(t'); diag chunks have global key
    offsets 128*p (slot 0) and 128*(p+2) (slot 1) within the chunk."""
    perm = _perm(p)
    sigma = perm[:QC] % QC  # within-chunk token offset pattern (j-independent)
    s = np.arange(P)[:, None]
    m = np.empty((P, 2, QC), np.float32)
    m[:, 0] = (sigma[None, :] - s - KC * p) >= 0
    m[:, 1] = (sigma[None, :] - s - KC * (p + 2)) >= 0
    return m


def _run(x, Wk, Wq, Wv, trace=False):
    from concourse.bass_utils import run_bass_kernel_spmd

    x = np.asarray(x, dtype=np.float32)
    Wk = np.asarray(Wk, dtype=np.float32)
    Wq = np.asarray(Wq, dtype=np.float32)
    Wv = np.asarray(Wv, dtype=np.float32)

    if COMPUTE == "f32r":
        conv = _round_f32r
    else:
        import ml_dtypes
        conv = lambda a: np.asarray(a, dtype=ml_dtypes.bfloat16)
    wkv = conv(np.concatenate([Wk.T, Wv.T], axis=1))
    wq0 = conv(np.concatenate([Wq.T, np.zeros((E, H), np.float32)], axis=1))
    masks = [conv(_make_masks(0)), conv(_make_masks(1))]
    perms = [_perm(0), _perm(1)]

    in_maps = []
    xTb = {}
    for c in range(8):
        b, p = c // 2, c % 2
        if (b, p) not in xTb:
            xTb[(b, p)] = conv(x[b].T[:, perms[p]])
        in_maps.append({"xTa": xTb[(b, p)], "wkv": wkv, "wq0": wq0,
                        "dmask": masks[p]})

    nc = _get_graph()
    res = run_bass_kernel_spmd(nc, in_maps, core_ids=list(range(8)), trace=trace)

    out = np.empty((B, T, H), dtype=np.float32)
    for b in range(B):
        o0 = res.results[2 * b]["o"].reshape(H + 1, T)
        o1 = res.results[2 * b + 1]["o"].reshape(H + 1, T)
        # p=1 columns are block-swapped; un-permute before merging
        o1 = o1[:, perms[1]]
        s = o0 + o1
        out[b] = (s[0:H] / s[H:H + 1]).T
    return out, res.exec_time_ns


def kernel(x, Wk, Wq, Wv):
    out, _ = _run(x, Wk, Wq, Wv)
    return out
